# revision 25
# baseline (speedup 1.0000x reference)
"""Trainium2 Bass kernel for nn_COINSEG_Contrastive_Loss.

Strategy (data-parallel over batch B=8, one batch element per NeuronCore):
  Per core, for its image b:
   - labels_down / old_pred pseudo-label built on-chip from strided loads of
     labels and outputs_old (nearest-down == subsample at stride 4).
   - features / features_old [256, 16384] streamed in [128, 2048] tiles,
     PE-transposed to [pixel, C] chunks of 128 pixels.
   - Per-pixel L2 norms via fused square+reduce; 1/norm folded into the
     per-class one-hot weights (so the big tensors are never rescaled).
   - Segment sums become PE matmuls: psum[21, 256] += onehot_w.T @ feat_t,
     accumulated across all 128 chunks. Counts accumulated on gpsimd.
  Host: sum the 8 cores' partial [21,256] sums + counts, then evaluate the
  tiny 21x42 contrastive loss exactly as the reference does.

Self-contained: only needs numpy/jax/concourse (the axon TRN2 runtime).
"""

import numpy as np

import concourse.bacc as bacc
import concourse.mybir as mybir
from concourse.tile import TileContext

F32 = mybir.dt.float32
F32R = mybir.dt.float32r
I32 = mybir.dt.int32
Alu = mybir.AluOpType
Act = mybir.ActivationFunctionType
Axis = mybir.AxisListType

N_CORES = 8
B, C, H, W = 8, 256, 128, 128
NPIX = H * W            # 16384 pixels per image (after nearest-down)
K = 21                  # num classes
CH = 16                 # old-model channels
N_CHUNK = 128           # 128 pixels per chunk (one downsampled row)
N_GROUP = 16            # 8 chunks per group
CHUNKS_PER_GROUP = 8
WTILE = 2048            # pixels per feature DMA tile
TEMPERATURE = 0.07
THRESHOLD = 0.7
NEG_BIG = 1e30


def build_nc(loop_iters: int = 1, n_groups: int = N_GROUP):
    """Build the per-core Bass program.

    loop_iters > 1 wraps the whole body in a For_i loop for timing; the
    outputs are iteration-invariant so correctness is unaffected.
    """
    nc = bacc.Bacc("TRN2", target_bir_lowering=False, debug=False)

    feat = nc.dram_tensor("feat", [C, NPIX], F32R, kind="ExternalInput")
    feat_old = nc.dram_tensor("feat_old", [C, NPIX], F32R, kind="ExternalInput")
    oo = nc.dram_tensor("oo", [CH, 4 * H, 4 * W], F32, kind="ExternalInput")
    lab = nc.dram_tensor("lab", [4 * H, 4 * W], I32, kind="ExternalInput")
    ident = nc.dram_tensor("ident", [128, 128], F32R, kind="ExternalInput")
    iota16 = nc.dram_tensor("iota16", [128, 128], F32, kind="ExternalInput")
    iota21 = nc.dram_tensor("iota21", [128, 8 * K], F32, kind="ExternalInput")

    out_sa = nc.dram_tensor("out_sa", [K, C], F32, kind="ExternalOutput")
    out_so = nc.dram_tensor("out_so", [K, C], F32, kind="ExternalOutput")
    out_cnt = nc.dram_tensor("out_cnt", [128, 8 * K], F32, kind="ExternalOutput")

    with TileContext(nc) as tc:
        with (
            tc.tile_pool(name="const", bufs=1) as constp,
            tc.tile_pool(name="fdma", bufs=8) as fdma,
            tc.tile_pool(name="fr", bufs=2) as frp,
            tc.tile_pool(name="scr", bufs=2) as scrp,
            tc.tile_pool(name="lblsml", bufs=3) as lbl,
            tc.tile_pool(name="lblbig", bufs=2) as lblb,
            tc.tile_pool(name="oneg", bufs=2) as onegp,
            tc.tile_pool(name="persist", bufs=1) as pers,
            tc.tile_pool(name="pairT", bufs=2, space="PSUM") as pairTp,
            tc.tile_pool(name="ooT", bufs=2, space="PSUM") as ooTp,
            tc.tile_pool(name="psacc", bufs=1, space="PSUM") as psacc,
        ):
            ident_t = constp.tile([128, 128], F32R)
            nc.sync.dma_start(out=ident_t[:], in_=ident.ap())
            iota16_t = constp.tile([128, 128], F32)
            nc.sync.dma_start(out=iota16_t[:], in_=iota16.ap())
            iota21_t = constp.tile([128, 8 * K], F32)
            nc.sync.dma_start(out=iota21_t[:], in_=iota21.ap())

            psum_a = psacc.tile([K, C], F32)
            psum_o = psacc.tile([K, C], F32)
            cnt = pers.tile([128, 8 * K], F32)

            def body(_iv=None):
                nc.vector.memset(cnt[:], 0.0)

                # ---- labels: rows 4h, then ::4 in w, cast to f32, transpose
                labr = lblb.tile([128, 4 * W], I32, tag="labr")
                nc.sync.dma_start(
                    out=labr[:],
                    in_=lab.ap().rearrange("(h s) w -> s h w", s=4)[0],
                )
                labf = lbl.tile([128, 128], F32, tag="labf")
                nc.vector.tensor_copy(
                    labf[:],
                    labr[:].rearrange("p (w s) -> p w s", s=4)[:, :, 0],
                )
                labT_ps = ooTp.tile([128, 128], F32, tag="ooT")
                nc.tensor.transpose(
                    labT_ps[:], labf[:], ident_t[:].bitcast(F32)
                )
                labT = pers.tile([128, 128], F32, tag="labT")
                nc.scalar.copy(labT[:], labT_ps[:])

                feat_tiles = {}

                def load_w(w):
                    if w in feat_tiles or w * WTILE >= n_groups * 1024:
                        return
                    tl = []
                    for src, half in (
                        (feat, 0), (feat, 1), (feat_old, 0), (feat_old, 1)
                    ):
                        t = fdma.tile([128, WTILE], F32R, tag="fdma")
                        nc.sync.dma_start(
                            out=t[:],
                            in_=src.ap()[
                                half * 128 : half * 128 + 128,
                                w * WTILE : (w + 1) * WTILE,
                            ],
                        )
                        tl.append(t)
                    feat_tiles[w] = tl

                def emit_matmuls(gg, wa, wo, fr):
                    for j in range(CHUNKS_PER_GROUP):
                        c = gg * CHUNKS_PER_GROUP + j
                        first = c == 0
                        last = c == n_groups * CHUNKS_PER_GROUP - 1
                        nc.tensor.matmul(
                            psum_a[:],
                            wa[:, K * j : K * j + K],
                            fr[:, 512 * j : 512 * j + 256],
                            start=first,
                            stop=last,
                        )
                        nc.tensor.matmul(
                            psum_o[:],
                            wo[:, K * j : K * j + K],
                            fr[:, 512 * j + 256 : 512 * j + 512],
                            start=first,
                            stop=last,
                        )

                pending = None
                load_w(0)
                for g in range(n_groups):
                    w = (g * 1024) // WTILE
                    load_w(w)
                    load_w(w + 1)  # prefetch
                    woff = (g * 1024) % WTILE
                    fa0, fa1, fo0, fo1 = feat_tiles[w]

                    # ---- old-model argmax pipeline for this group's 8 rows
                    oo_pack = lblb.tile([128, 4 * W], F32, tag="oopack")
                    nc.sync.dma_start(
                        out=oo_pack[:],
                        in_=oo.ap().rearrange(
                            "c (g j s) w -> s g j c w", s=4, j=8
                        )[0, g],
                    )
                    oo_g = lbl.tile([128, 128], F32, tag="oog")
                    nc.gpsimd.tensor_copy(
                        oo_g[:],
                        oo_pack[:].rearrange("p (w s) -> p w s", s=4)[:, :, 0],
                    )
                    ooT_ps = ooTp.tile([128, 128], F32, tag="ooT")
                    nc.tensor.transpose(
                        ooT_ps[:], oo_g[:], ident_t[:].bitcast(F32)
                    )
                    oot = lbl.tile([128, 128], F32, tag="oot")
                    nc.scalar.copy(oot[:], ooT_ps[:])
                    oot3 = oot[:].rearrange("p (j c) -> p j c", c=CH)

                    m8 = lbl.tile([128, 8], F32, tag="m8")
                    nc.vector.tensor_reduce(m8[:], oot3, Axis.X, Alu.max)
                    ge = lbl.tile([128, 128], F32, tag="ge")
                    nc.vector.tensor_tensor(
                        ge[:].rearrange("p (j c) -> p j c", c=CH),
                        oot3,
                        m8[:].unsqueeze(2).broadcast_to([128, 8, CH]),
                        Alu.is_ge,
                    )
                    ti = lbl.tile([128, 128], F32, tag="ti")
                    nc.gpsimd.tensor_tensor(ti[:], ge[:], iota16_t[:], Alu.mult)
                    idx8 = lbl.tile([128, 8], F32, tag="idx8")
                    nc.vector.tensor_reduce(
                        idx8[:],
                        ti[:].rearrange("p (j c) -> p j c", c=CH),
                        Axis.X,
                        Alu.max,
                    )
                    ge7 = lbl.tile([128, 8], F32, tag="ge7")
                    nc.gpsimd.tensor_scalar(
                        ge7[:], m8[:], THRESHOLD, None, Alu.is_ge
                    )
                    old8 = lbl.tile([128, 8], F32, tag="old8")
                    nc.gpsimd.tensor_tensor(old8[:], ge7[:], idx8[:], Alu.mult)
                    labc = labT[:, 8 * g : 8 * g + 8]
                    isz = lbl.tile([128, 8], F32, tag="isz")
                    nc.gpsimd.tensor_scalar(isz[:], labc, 0.0, None, Alu.is_equal)
                    tmp8 = lbl.tile([128, 8], F32, tag="tmp8")
                    nc.gpsimd.tensor_tensor(tmp8[:], old8[:], isz[:], Alu.mult)
                    ps8 = lbl.tile([128, 8], F32, tag="ps8")
                    nc.gpsimd.tensor_tensor(ps8[:], labc, tmp8[:], Alu.add)

                    oneh = onegp.tile([128, 8 * K], F32, tag="oneh")
                    nc.vector.tensor_tensor(
                        oneh[:].rearrange("p (j k) -> p j k", k=K),
                        iota21_t[:].rearrange("p (j k) -> p j k", k=K),
                        ps8[:].unsqueeze(2).broadcast_to([128, 8, K]),
                        Alu.is_equal,
                    )
                    nc.gpsimd.tensor_tensor(cnt[:], cnt[:], oneh[:], Alu.add)

                    # ---- features: transpose, evac, norms, weights, matmuls
                    fr = frp.tile([128, 4096], F32R, tag="fr")
                    n2 = lbl.tile([128, 16], F32, tag="n2")
                    for p in range(4):  # chunk pairs within group
                        pairT = pairTp.tile([128, 1024], F32R, tag="pairT")
                        for q in range(2):  # chunk in pair
                            j = 2 * p + q
                            off = woff + j * 128
                            o = 512 * q
                            nc.tensor.transpose(
                                pairT[:, o : o + 128],
                                fa0[:, off : off + 128],
                                ident_t[:],
                            )
                            nc.tensor.transpose(
                                pairT[:, o + 128 : o + 256],
                                fa1[:, off : off + 128],
                                ident_t[:],
                            )
                            nc.tensor.transpose(
                                pairT[:, o + 256 : o + 384],
                                fo0[:, off : off + 128],
                                ident_t[:],
                            )
                            nc.tensor.transpose(
                                pairT[:, o + 384 : o + 512],
                                fo1[:, off : off + 128],
                                ident_t[:],
                            )
                        # evacuate the pair (2 chunks x [fa|fo]) in one pass
                        # on ACT (large copies amortize its fixed overheads)
                        nc.scalar.copy(
                            fr[:, 1024 * p : 1024 * p + 1024], pairT[:]
                        )
                        # per-chunk squared-norm accumulations via fused
                        # square+row-sum STT on DVE
                        for q in range(2):
                            j = 2 * p + q
                            for t in range(2):  # 0=fa, 1=fo
                                src = fr[
                                    :, 512 * j + 256 * t : 512 * j + 256 * t + 256
                                ].bitcast(F32)
                                scrv = scrp.tile([128, 256], F32, tag="scrv")
                                nc.vector.scalar_tensor_tensor(
                                    out=scrv[:],
                                    in0=src,
                                    scalar=1.0,
                                    in1=src,
                                    op0=Alu.mult,
                                    op1=Alu.mult,
                                    accum_out=n2[:, 2 * j + t : 2 * j + t + 1],
                                )

                    # rnorm = 1/sqrt(n2): ACT sqrt + accurate DVE reciprocal
                    nrm = lbl.tile([128, 16], F32, tag="nrm")
                    nc.scalar.sqrt(nrm[:], n2[:])
                    rn = lbl.tile([128, 16], F32, tag="rn")
                    nc.vector.reciprocal(rn[:], nrm[:])

                    rn3 = rn[:].rearrange("p (j t) -> p j t", t=2)
                    wa = onegp.tile([128, 8 * K], F32R, tag="wa")
                    nc.vector.tensor_tensor(
                        wa[:].rearrange("p (j k) -> p j k", k=K),
                        oneh[:].bitcast(F32R).rearrange("p (j k) -> p j k", k=K),
                        rn3[:, :, 0]
                        .bitcast(F32R)
                        .unsqueeze(2)
                        .broadcast_to([128, 8, K]),
                        Alu.mult,
                    )
                    wo = onegp.tile([128, 8 * K], F32R, tag="wo")
                    nc.vector.tensor_tensor(
                        wo[:].rearrange("p (j k) -> p j k", k=K),
                        oneh[:].bitcast(F32R).rearrange("p (j k) -> p j k", k=K),
                        rn3[:, :, 1]
                        .bitcast(F32R)
                        .unsqueeze(2)
                        .broadcast_to([128, 8, K]),
                        Alu.mult,
                    )

                    # matmuls run one group late so the PE never waits on
                    # the evac->norm->weights chain of the current group
                    if pending is not None:
                        emit_matmuls(*pending)
                    pending = (g, wa, wo, fr)

                    if woff + 1024 >= WTILE:
                        feat_tiles.pop(w, None)

                if pending is not None:
                    emit_matmuls(*pending)

                # ---- outputs (PSUM must bounce through SBUF for DMA)
                sa_s = pers.tile([K, C], F32, tag="sa_s")
                so_s = pers.tile([K, C], F32, tag="so_s")
                nc.vector.tensor_copy(sa_s[:], psum_a[:])
                nc.vector.tensor_copy(so_s[:], psum_o[:])
                nc.sync.dma_start(out=out_sa.ap(), in_=sa_s[:])
                nc.sync.dma_start(out=out_so.ap(), in_=so_s[:])
                nc.sync.dma_start(out=out_cnt.ap(), in_=cnt[:])

            if loop_iters == 1:
                body()
            elif loop_iters < 0:  # python-unrolled (TimelineSim can't For_i)
                for _ in range(-loop_iters):
                    body()
            else:
                with tc.For_i(0, loop_iters, 1) as iv:
                    body(iv)

    nc.compile()
    return nc


# ---------------------------------------------------------------------------
# SPMD runner (cached-jit variant of bass2jax.run_bass_via_pjrt)
# ---------------------------------------------------------------------------
class _SpmdRunner:
    def __init__(self, nc, n_cores):
        import jax
        from jax.sharding import Mesh, PartitionSpec
        from jax.experimental.shard_map import shard_map
        from concourse.bass2jax import (
            _bass_exec_p,
            install_neuronx_cc_hook,
            partition_id_tensor,
        )

        install_neuronx_cc_hook()
        self.jax = jax
        self.n_cores = n_cores
        in_names, out_names, out_avals = [], [], []
        for alloc in nc.m.functions[0].allocations:
            if not isinstance(alloc, mybir.MemoryLocationSet):
                continue
            name = alloc.memorylocations[0].name
            if alloc.kind == "ExternalInput":
                in_names.append(name)
            elif alloc.kind == "ExternalOutput":
                out_names.append(name)
                out_avals.append(
                    jax.core.ShapedArray(
                        tuple(alloc.tensor_shape), mybir.dt.np(alloc.dtype)
                    )
                )
        part_name = nc.partition_id_tensor.name if nc.partition_id_tensor else None
        if part_name in in_names:
            in_names.remove(part_name)
        self.in_names, self.out_names, self.out_avals = (
            in_names,
            out_names,
            out_avals,
        )
        all_names = tuple(in_names + out_names)
        if part_name is not None:
            all_names = all_names + (part_name,)

        def _body(*args):
            operands = list(args)
            if part_name is not None:
                operands.append(partition_id_tensor())
            return tuple(
                _bass_exec_p.bind(
                    *operands,
                    out_avals=tuple(out_avals),
                    in_names=all_names,
                    out_names=tuple(out_names),
                    lowering_input_output_aliases=(),
                    sim_require_finite=True,
                    sim_require_nnan=True,
                    nc=nc,
                )
            )

        devices = jax.devices()[:n_cores]
        self.mesh = Mesh(np.asarray(devices), ("core",))
        n_args = len(in_names) + len(out_names)
        self.fn = jax.jit(
            shard_map(
                _body,
                mesh=self.mesh,
                in_specs=(PartitionSpec("core"),) * n_args,
                out_specs=(PartitionSpec("core"),) * len(out_names),
                check_rep=False,
            ),
            keep_unused=True,
        )

    def stage(self, in_maps):
        import jax
        from jax.sharding import NamedSharding, PartitionSpec

        n = self.n_cores
        concat_in = [
            np.concatenate([np.asarray(in_maps[c][k]) for c in range(n)], axis=0)
            for k in self.in_names
        ]
        concat_zero = [
            np.zeros((n * a.shape[0], *a.shape[1:]), a.dtype)
            for a in self.out_avals
        ]
        sh = NamedSharding(self.mesh, PartitionSpec("core"))
        self._args = [jax.device_put(a, sh) for a in concat_in + concat_zero]

    def execute(self):
        out = self.fn(*self._args)
        self.jax.block_until_ready(out)
        return out

    def results(self, out):
        n = self.n_cores
        res = []
        for c in range(n):
            d = {}
            for i, k in enumerate(self.out_names):
                a = np.asarray(out[i])
                per = a.shape[0] // n
                d[k] = a[c * per : (c + 1) * per]
            res.append(d)
        return res


def make_const_inputs():
    ident = np.eye(128, dtype=np.float32)
    iota16 = np.tile(np.arange(16, dtype=np.float32), 8)[None, :].repeat(128, 0)
    iota21 = np.tile(np.arange(K, dtype=np.float32), 8)[None, :].repeat(128, 0)
    return ident, np.ascontiguousarray(iota16), np.ascontiguousarray(iota21)


def make_in_maps(labels, features_old, features, outputs_old):
    ident, iota16, iota21 = make_const_inputs()
    labels = np.asarray(labels, dtype=np.int32)
    features = np.asarray(features, dtype=np.float32)
    features_old = np.asarray(features_old, dtype=np.float32)
    outputs_old = np.asarray(outputs_old, dtype=np.float32)
    in_maps = []
    for b in range(N_CORES):
        in_maps.append(
            {
                "feat": np.ascontiguousarray(features[b].reshape(C, NPIX)),
                "feat_old": np.ascontiguousarray(
                    features_old[b].reshape(C, NPIX)
                ),
                "oo": np.ascontiguousarray(outputs_old[b]),
                "lab": np.ascontiguousarray(labels[b]),
                "ident": ident,
                "iota16": iota16,
                "iota21": iota21,
            }
        )
    return in_maps


def host_finish(counts, sum_a, sum_o):
    """Replicates the reference's tiny [K, 2K] contrastive computation."""
    counts = counts.astype(np.float64)
    sum_a = sum_a.astype(np.float64)
    sum_o = sum_o.astype(np.float64)
    present = counts > 0
    denom = np.where(present, counts, 1.0)[:, None]
    anc = np.where(present[:, None], sum_a / denom, 0.0)
    con = np.where(present[:, None], sum_o / denom, 0.0)
    contrast = np.concatenate([anc, con], axis=0)

    eye = np.eye(K)
    rowp = present.astype(np.float64)
    colp = np.concatenate([rowp, rowp])
    pos_mask = (
        np.concatenate([np.zeros((K, K)), eye], axis=1)
        * rowp[:, None]
        * colp[None, :]
    )
    neg_mask = (
        (1.0 - np.concatenate([eye, eye], axis=1))
        * rowp[:, None]
        * colp[None, :]
    )

    adc = (anc @ contrast.T) / TEMPERATURE
    neg = np.sum(np.exp(adc) * neg_mask, axis=1, keepdims=True)
    logits_max = np.max(
        np.where(colp[None, :] > 0, adc, -NEG_BIG), axis=1, keepdims=True
    )
    shifted = adc - logits_max
    pos_contrast = shifted * pos_mask - np.log(np.exp(shifted) + neg) * pos_mask

    num = pos_mask.sum(axis=1)
    valid = num > 0
    row_loss = -pos_contrast.sum(axis=1) / np.where(valid, num, 1.0)
    loss = np.sum(np.where(valid, row_loss, 0.0)) / max(valid.sum(), 1.0)
    return np.float32(loss)


def combine_results(results):
    counts = np.zeros(K, dtype=np.float64)
    sum_a = np.zeros((K, C), dtype=np.float64)
    sum_o = np.zeros((K, C), dtype=np.float64)
    for r in results:
        counts += r["out_cnt"].astype(np.float64).sum(0).reshape(8, K).sum(0)
        sum_a += r["out_sa"].astype(np.float64)
        sum_o += r["out_so"].astype(np.float64)
    return counts, sum_a, sum_o


_RUNNER = None


def _get_runner():
    global _RUNNER
    if _RUNNER is None:
        nc = build_nc()
        _RUNNER = _SpmdRunner(nc, N_CORES)
    return _RUNNER


def kernel(
    labels,
    features_old,
    features,
    outputs_old,
    outputs=None,
    prototypes=None,
    num_class=21,
    num_old_class=16,
    num_new_class=5,
    epoch=1,
    train_step=1,
    len_epoch=100,
):
    r = _get_runner()
    r.stage(make_in_maps(labels, features_old, features, outputs_old))
    out = r.execute()
    counts, sum_a, sum_o = combine_results(r.results(out))
    return host_finish(counts, sum_a, sum_o)



# revision 29
# speedup vs baseline: 1.2654x; 1.2654x over previous
"""Trainium2 Bass kernel for nn_COINSEG_Contrastive_Loss.

Strategy (data-parallel over batch B=8, one batch element per NeuronCore):
  Per core, for its image b:
   - labels_down / old_pred pseudo-label built on-chip from strided loads of
     labels and outputs_old (nearest-down == subsample at stride 4).
   - features / features_old [256, 16384] streamed in [128, 2048] tiles,
     PE-transposed to [pixel, C] chunks of 128 pixels.
   - Per-pixel L2 norms via fused square+reduce; 1/norm folded into the
     per-class one-hot weights (so the big tensors are never rescaled).
   - Segment sums become PE matmuls: psum[21, 256] += onehot_w.T @ feat_t,
     accumulated across all 128 chunks. Counts accumulated on gpsimd.
  Host: sum the 8 cores' partial [21,256] sums + counts, then evaluate the
  tiny 21x42 contrastive loss exactly as the reference does.

Self-contained: only needs numpy/jax/concourse (the axon TRN2 runtime).
"""

import numpy as np

import concourse.bacc as bacc
import concourse.mybir as mybir
from concourse.tile import TileContext

F32 = mybir.dt.float32
F32R = mybir.dt.float32r
I32 = mybir.dt.int32
Alu = mybir.AluOpType
Act = mybir.ActivationFunctionType
Axis = mybir.AxisListType

N_CORES = 8
B, C, H, W = 8, 256, 128, 128
NPIX = H * W            # 16384 pixels per image (after nearest-down)
K = 21                  # num classes
CH = 16                 # old-model channels
N_CHUNK = 128           # 128 pixels per chunk (one downsampled row)
N_GROUP = 16            # 8 chunks per group
CHUNKS_PER_GROUP = 8
WTILE = 1024            # pixels per feature DMA tile (one group's worth)
TEMPERATURE = 0.07
THRESHOLD = 0.7
NEG_BIG = 1e30


def build_nc(loop_iters: int = 1, n_groups: int = N_GROUP):
    """Build the per-core Bass program.

    loop_iters > 1 wraps the whole body in a For_i loop for timing; the
    outputs are iteration-invariant so correctness is unaffected.
    """
    nc = bacc.Bacc("TRN2", target_bir_lowering=False, debug=False)

    feat = nc.dram_tensor("feat", [C, NPIX], F32R, kind="ExternalInput")
    feat_old = nc.dram_tensor("feat_old", [C, NPIX], F32R, kind="ExternalInput")
    oo = nc.dram_tensor("oo", [CH, 4 * H, 4 * W], F32, kind="ExternalInput")
    lab = nc.dram_tensor("lab", [4 * H, 4 * W], I32, kind="ExternalInput")
    ident = nc.dram_tensor("ident", [128, 128], F32R, kind="ExternalInput")
    iota16 = nc.dram_tensor("iota16", [128, 128], F32, kind="ExternalInput")
    iota21 = nc.dram_tensor("iota21", [128, 8 * K], F32, kind="ExternalInput")

    out_sa = nc.dram_tensor("out_sa", [K, C], F32, kind="ExternalOutput")
    out_so = nc.dram_tensor("out_so", [K, C], F32, kind="ExternalOutput")
    out_cnt = nc.dram_tensor("out_cnt", [128, 8 * K], F32, kind="ExternalOutput")

    with TileContext(nc) as tc:
        with (
            tc.tile_pool(name="const", bufs=1) as constp,
            tc.tile_pool(name="fdma", bufs=8) as fdma,
            tc.tile_pool(name="fr", bufs=2) as frp,
            tc.tile_pool(name="scr", bufs=2) as scrp,
            tc.tile_pool(name="lblsml", bufs=3) as lbl,
            tc.tile_pool(name="lblbig", bufs=2) as lblb,
            tc.tile_pool(name="oneg", bufs=2) as onegp,
            tc.tile_pool(name="persist", bufs=1) as pers,
            tc.tile_pool(name="pairT", bufs=2, space="PSUM") as pairTp,
            tc.tile_pool(name="ooT", bufs=2, space="PSUM") as ooTp,
            tc.tile_pool(name="psacc", bufs=1, space="PSUM") as psacc,
        ):
            ident_t = constp.tile([128, 128], F32R)
            nc.sync.dma_start(out=ident_t[:], in_=ident.ap())
            iota16_t = constp.tile([128, 128], F32)
            nc.sync.dma_start(out=iota16_t[:], in_=iota16.ap())
            iota21_t = constp.tile([128, 8 * K], F32)
            nc.sync.dma_start(out=iota21_t[:], in_=iota21.ap())

            psum_a = psacc.tile([K, C], F32)
            psum_o = psacc.tile([K, C], F32)
            cnt = pers.tile([128, 8 * K], F32)

            def body(_iv=None):
                nc.vector.memset(cnt[:], 0.0)

                # ---- labels: rows 4h, then ::4 in w, cast to f32, transpose
                labr = lblb.tile([128, 4 * W], I32, tag="labr")
                nc.scalar.dma_start(
                    out=labr[:],
                    in_=lab.ap().rearrange("(h s) w -> s h w", s=4)[0],
                )
                labf = lbl.tile([128, 128], F32, tag="labf")
                nc.vector.tensor_copy(
                    labf[:],
                    labr[:].rearrange("p (w s) -> p w s", s=4)[:, :, 0],
                )
                labT_ps = ooTp.tile([128, 128], F32, tag="ooT")
                nc.tensor.transpose(
                    labT_ps[:], labf[:], ident_t[:].bitcast(F32)
                )
                labT = pers.tile([128, 128], F32, tag="labT")
                nc.scalar.copy(labT[:], labT_ps[:])

                feat_tiles = {}

                def load_w(w):
                    if w in feat_tiles or w * WTILE >= n_groups * 1024:
                        return
                    tl = []
                    for src, half in (
                        (feat, 0), (feat, 1), (feat_old, 0), (feat_old, 1)
                    ):
                        t = fdma.tile([128, WTILE], F32R, tag="fdma")
                        nc.sync.dma_start(
                            out=t[:],
                            in_=src.ap()[
                                half * 128 : half * 128 + 128,
                                w * WTILE : (w + 1) * WTILE,
                            ],
                        )
                        tl.append(t)
                    feat_tiles[w] = tl

                def emit_matmuls(gg, wa, wo, fr):
                    for j in range(CHUNKS_PER_GROUP):
                        c = gg * CHUNKS_PER_GROUP + j
                        first = c == 0
                        last = c == n_groups * CHUNKS_PER_GROUP - 1
                        nc.tensor.matmul(
                            psum_a[:],
                            wa[:, K * j : K * j + K],
                            fr[:, 512 * j : 512 * j + 256],
                            start=first,
                            stop=last,
                        )
                        nc.tensor.matmul(
                            psum_o[:],
                            wo[:, K * j : K * j + K],
                            fr[:, 512 * j + 256 : 512 * j + 512],
                            start=first,
                            stop=last,
                        )

                pending = None
                load_w(0)
                for g in range(n_groups):
                    w = (g * 1024) // WTILE
                    load_w(w)
                    load_w(w + 1)  # prefetch
                    woff = (g * 1024) % WTILE
                    fa0, fa1, fo0, fo1 = feat_tiles[w]

                    # ---- old-model argmax pipeline for this group's 8 rows
                    oo_pack = lblb.tile([128, 4 * W], F32, tag="oopack")
                    nc.scalar.dma_start(
                        out=oo_pack[:],
                        in_=oo.ap().rearrange(
                            "c (g j s) w -> s g j c w", s=4, j=8
                        )[0, g],
                    )
                    oo_g = lbl.tile([128, 128], F32, tag="oog")
                    nc.gpsimd.tensor_copy(
                        oo_g[:],
                        oo_pack[:].rearrange("p (w s) -> p w s", s=4)[:, :, 0],
                    )
                    ooT_ps = ooTp.tile([128, 128], F32, tag="ooT")
                    nc.tensor.transpose(
                        ooT_ps[:], oo_g[:], ident_t[:].bitcast(F32)
                    )
                    oot = lbl.tile([128, 128], F32, tag="oot")
                    nc.scalar.copy(oot[:], ooT_ps[:])
                    oot3 = oot[:].rearrange("p (j c) -> p j c", c=CH)

                    m8 = lbl.tile([128, 8], F32, tag="m8")
                    nc.vector.tensor_reduce(m8[:], oot3, Axis.X, Alu.max)
                    ge = lbl.tile([128, 128], F32, tag="ge")
                    nc.vector.tensor_tensor(
                        ge[:].rearrange("p (j c) -> p j c", c=CH),
                        oot3,
                        m8[:].unsqueeze(2).broadcast_to([128, 8, CH]),
                        Alu.is_ge,
                    )
                    ti = lbl.tile([128, 128], F32, tag="ti")
                    nc.gpsimd.tensor_tensor(ti[:], ge[:], iota16_t[:], Alu.mult)
                    idx8 = lbl.tile([128, 8], F32, tag="idx8")
                    nc.vector.tensor_reduce(
                        idx8[:],
                        ti[:].rearrange("p (j c) -> p j c", c=CH),
                        Axis.X,
                        Alu.max,
                    )
                    ge7 = lbl.tile([128, 8], F32, tag="ge7")
                    nc.gpsimd.tensor_scalar(
                        ge7[:], m8[:], THRESHOLD, None, Alu.is_ge
                    )
                    old8 = lbl.tile([128, 8], F32, tag="old8")
                    nc.gpsimd.tensor_tensor(old8[:], ge7[:], idx8[:], Alu.mult)
                    labc = labT[:, 8 * g : 8 * g + 8]
                    isz = lbl.tile([128, 8], F32, tag="isz")
                    nc.gpsimd.tensor_scalar(isz[:], labc, 0.0, None, Alu.is_equal)
                    tmp8 = lbl.tile([128, 8], F32, tag="tmp8")
                    nc.gpsimd.tensor_tensor(tmp8[:], old8[:], isz[:], Alu.mult)
                    ps8 = lbl.tile([128, 8], F32, tag="ps8")
                    nc.gpsimd.tensor_tensor(ps8[:], labc, tmp8[:], Alu.add)

                    oneh = onegp.tile([128, 8 * K], F32, tag="oneh")
                    nc.vector.tensor_tensor(
                        oneh[:].rearrange("p (j k) -> p j k", k=K),
                        iota21_t[:].rearrange("p (j k) -> p j k", k=K),
                        ps8[:].unsqueeze(2).broadcast_to([128, 8, K]),
                        Alu.is_equal,
                    )
                    nc.gpsimd.tensor_tensor(cnt[:], cnt[:], oneh[:], Alu.add)

                    # ---- features: transpose, evac, norms, weights, matmuls
                    fr = frp.tile([128, 4096], F32R, tag="fr")
                    n2 = lbl.tile([128, 16], F32, tag="n2")
                    for p in range(4):  # chunk pairs within group
                        pairT = pairTp.tile([128, 1024], F32R, tag="pairT")
                        for q in range(2):  # chunk in pair
                            j = 2 * p + q
                            off = woff + j * 128
                            o = 512 * q
                            nc.tensor.transpose(
                                pairT[:, o : o + 128],
                                fa0[:, off : off + 128],
                                ident_t[:],
                            )
                            nc.tensor.transpose(
                                pairT[:, o + 128 : o + 256],
                                fa1[:, off : off + 128],
                                ident_t[:],
                            )
                            nc.tensor.transpose(
                                pairT[:, o + 256 : o + 384],
                                fo0[:, off : off + 128],
                                ident_t[:],
                            )
                            nc.tensor.transpose(
                                pairT[:, o + 384 : o + 512],
                                fo1[:, off : off + 128],
                                ident_t[:],
                            )
                        # evacuate the pair (2 chunks x [fa|fo]) in one pass
                        # on ACT (large copies amortize its fixed overheads)
                        nc.scalar.copy(
                            fr[:, 1024 * p : 1024 * p + 1024], pairT[:]
                        )
                        # per-chunk squared-norm accumulations: fused STT on
                        # DVE for 14/16, Square+accum on ACT for 2/16 so both
                        # engines sit just under the DMA roofline
                        for q in range(2):
                            j = 2 * p + q
                            for t in range(2):  # 0=fa, 1=fo
                                src = fr[
                                    :, 512 * j + 256 * t : 512 * j + 256 * t + 256
                                ].bitcast(F32)
                                acc = n2[:, 2 * j + t : 2 * j + t + 1]
                                if p == 3 and t == 1:
                                    scra = scrp.tile([128, 256], F32, tag="scra")
                                    nc.scalar.activation(
                                        scra[:], src, Act.Square, accum_out=acc
                                    )
                                else:
                                    scrv = scrp.tile([128, 256], F32, tag="scrv")
                                    nc.vector.scalar_tensor_tensor(
                                        out=scrv[:],
                                        in0=src,
                                        scalar=1.0,
                                        in1=src,
                                        op0=Alu.mult,
                                        op1=Alu.mult,
                                        accum_out=acc,
                                    )

                    # rnorm = 1/sqrt(n2): ACT sqrt + accurate DVE reciprocal
                    nrm = lbl.tile([128, 16], F32, tag="nrm")
                    nc.scalar.sqrt(nrm[:], n2[:])
                    rn = lbl.tile([128, 16], F32, tag="rn")
                    nc.vector.reciprocal(rn[:], nrm[:])

                    rn3 = rn[:].rearrange("p (j t) -> p j t", t=2)
                    wa = onegp.tile([128, 8 * K], F32R, tag="wa")
                    nc.vector.tensor_tensor(
                        wa[:].rearrange("p (j k) -> p j k", k=K),
                        oneh[:].bitcast(F32R).rearrange("p (j k) -> p j k", k=K),
                        rn3[:, :, 0]
                        .bitcast(F32R)
                        .unsqueeze(2)
                        .broadcast_to([128, 8, K]),
                        Alu.mult,
                    )
                    wo = onegp.tile([128, 8 * K], F32R, tag="wo")
                    nc.vector.tensor_tensor(
                        wo[:].rearrange("p (j k) -> p j k", k=K),
                        oneh[:].bitcast(F32R).rearrange("p (j k) -> p j k", k=K),
                        rn3[:, :, 1]
                        .bitcast(F32R)
                        .unsqueeze(2)
                        .broadcast_to([128, 8, K]),
                        Alu.mult,
                    )

                    # matmuls run one group late so the PE never waits on
                    # the evac->norm->weights chain of the current group
                    if pending is not None:
                        emit_matmuls(*pending)
                    pending = (g, wa, wo, fr)

                    if woff + 1024 >= WTILE:
                        feat_tiles.pop(w, None)

                if pending is not None:
                    emit_matmuls(*pending)

                # ---- outputs (PSUM must bounce through SBUF for DMA)
                sa_s = pers.tile([K, C], F32, tag="sa_s")
                so_s = pers.tile([K, C], F32, tag="so_s")
                nc.vector.tensor_copy(sa_s[:], psum_a[:])
                nc.vector.tensor_copy(so_s[:], psum_o[:])
                nc.sync.dma_start(out=out_sa.ap(), in_=sa_s[:])
                nc.sync.dma_start(out=out_so.ap(), in_=so_s[:])
                nc.sync.dma_start(out=out_cnt.ap(), in_=cnt[:])

            if loop_iters == 1:
                body()
            elif loop_iters < 0:  # python-unrolled (TimelineSim can't For_i)
                for _ in range(-loop_iters):
                    body()
            else:
                with tc.For_i(0, loop_iters, 1) as iv:
                    body(iv)

    nc.compile()
    return nc


# ---------------------------------------------------------------------------
# SPMD runner (cached-jit variant of bass2jax.run_bass_via_pjrt)
# ---------------------------------------------------------------------------
class _SpmdRunner:
    def __init__(self, nc, n_cores):
        import jax
        from jax.sharding import Mesh, PartitionSpec
        from jax.experimental.shard_map import shard_map
        from concourse.bass2jax import (
            _bass_exec_p,
            install_neuronx_cc_hook,
            partition_id_tensor,
        )

        install_neuronx_cc_hook()
        self.jax = jax
        self.n_cores = n_cores
        in_names, out_names, out_avals = [], [], []
        for alloc in nc.m.functions[0].allocations:
            if not isinstance(alloc, mybir.MemoryLocationSet):
                continue
            name = alloc.memorylocations[0].name
            if alloc.kind == "ExternalInput":
                in_names.append(name)
            elif alloc.kind == "ExternalOutput":
                out_names.append(name)
                out_avals.append(
                    jax.core.ShapedArray(
                        tuple(alloc.tensor_shape), mybir.dt.np(alloc.dtype)
                    )
                )
        part_name = nc.partition_id_tensor.name if nc.partition_id_tensor else None
        if part_name in in_names:
            in_names.remove(part_name)
        self.in_names, self.out_names, self.out_avals = (
            in_names,
            out_names,
            out_avals,
        )
        all_names = tuple(in_names + out_names)
        if part_name is not None:
            all_names = all_names + (part_name,)

        def _body(*args):
            operands = list(args)
            if part_name is not None:
                operands.append(partition_id_tensor())
            return tuple(
                _bass_exec_p.bind(
                    *operands,
                    out_avals=tuple(out_avals),
                    in_names=all_names,
                    out_names=tuple(out_names),
                    lowering_input_output_aliases=(),
                    sim_require_finite=True,
                    sim_require_nnan=True,
                    nc=nc,
                )
            )

        devices = jax.devices()[:n_cores]
        self.mesh = Mesh(np.asarray(devices), ("core",))
        n_args = len(in_names) + len(out_names)
        self.fn = jax.jit(
            shard_map(
                _body,
                mesh=self.mesh,
                in_specs=(PartitionSpec("core"),) * n_args,
                out_specs=(PartitionSpec("core"),) * len(out_names),
                check_rep=False,
            ),
            keep_unused=True,
        )

    def stage(self, in_maps):
        import jax
        from jax.sharding import NamedSharding, PartitionSpec

        n = self.n_cores
        concat_in = [
            np.concatenate([np.asarray(in_maps[c][k]) for c in range(n)], axis=0)
            for k in self.in_names
        ]
        concat_zero = [
            np.zeros((n * a.shape[0], *a.shape[1:]), a.dtype)
            for a in self.out_avals
        ]
        sh = NamedSharding(self.mesh, PartitionSpec("core"))
        self._args = [jax.device_put(a, sh) for a in concat_in + concat_zero]

    def execute(self):
        out = self.fn(*self._args)
        self.jax.block_until_ready(out)
        return out

    def results(self, out):
        n = self.n_cores
        res = []
        for c in range(n):
            d = {}
            for i, k in enumerate(self.out_names):
                a = np.asarray(out[i])
                per = a.shape[0] // n
                d[k] = a[c * per : (c + 1) * per]
            res.append(d)
        return res


def make_const_inputs():
    ident = np.eye(128, dtype=np.float32)
    iota16 = np.tile(np.arange(16, dtype=np.float32), 8)[None, :].repeat(128, 0)
    iota21 = np.tile(np.arange(K, dtype=np.float32), 8)[None, :].repeat(128, 0)
    return ident, np.ascontiguousarray(iota16), np.ascontiguousarray(iota21)


def make_in_maps(labels, features_old, features, outputs_old):
    ident, iota16, iota21 = make_const_inputs()
    labels = np.asarray(labels, dtype=np.int32)
    features = np.asarray(features, dtype=np.float32)
    features_old = np.asarray(features_old, dtype=np.float32)
    outputs_old = np.asarray(outputs_old, dtype=np.float32)
    in_maps = []
    for b in range(N_CORES):
        in_maps.append(
            {
                "feat": np.ascontiguousarray(features[b].reshape(C, NPIX)),
                "feat_old": np.ascontiguousarray(
                    features_old[b].reshape(C, NPIX)
                ),
                "oo": np.ascontiguousarray(outputs_old[b]),
                "lab": np.ascontiguousarray(labels[b]),
                "ident": ident,
                "iota16": iota16,
                "iota21": iota21,
            }
        )
    return in_maps


def host_finish(counts, sum_a, sum_o):
    """Replicates the reference's tiny [K, 2K] contrastive computation."""
    counts = counts.astype(np.float64)
    sum_a = sum_a.astype(np.float64)
    sum_o = sum_o.astype(np.float64)
    present = counts > 0
    denom = np.where(present, counts, 1.0)[:, None]
    anc = np.where(present[:, None], sum_a / denom, 0.0)
    con = np.where(present[:, None], sum_o / denom, 0.0)
    contrast = np.concatenate([anc, con], axis=0)

    eye = np.eye(K)
    rowp = present.astype(np.float64)
    colp = np.concatenate([rowp, rowp])
    pos_mask = (
        np.concatenate([np.zeros((K, K)), eye], axis=1)
        * rowp[:, None]
        * colp[None, :]
    )
    neg_mask = (
        (1.0 - np.concatenate([eye, eye], axis=1))
        * rowp[:, None]
        * colp[None, :]
    )

    adc = (anc @ contrast.T) / TEMPERATURE
    neg = np.sum(np.exp(adc) * neg_mask, axis=1, keepdims=True)
    logits_max = np.max(
        np.where(colp[None, :] > 0, adc, -NEG_BIG), axis=1, keepdims=True
    )
    shifted = adc - logits_max
    pos_contrast = shifted * pos_mask - np.log(np.exp(shifted) + neg) * pos_mask

    num = pos_mask.sum(axis=1)
    valid = num > 0
    row_loss = -pos_contrast.sum(axis=1) / np.where(valid, num, 1.0)
    loss = np.sum(np.where(valid, row_loss, 0.0)) / max(valid.sum(), 1.0)
    return np.float32(loss)


def combine_results(results):
    counts = np.zeros(K, dtype=np.float64)
    sum_a = np.zeros((K, C), dtype=np.float64)
    sum_o = np.zeros((K, C), dtype=np.float64)
    for r in results:
        counts += r["out_cnt"].astype(np.float64).sum(0).reshape(8, K).sum(0)
        sum_a += r["out_sa"].astype(np.float64)
        sum_o += r["out_so"].astype(np.float64)
    return counts, sum_a, sum_o


_RUNNER = None


def _get_runner():
    global _RUNNER
    if _RUNNER is None:
        nc = build_nc()
        _RUNNER = _SpmdRunner(nc, N_CORES)
    return _RUNNER


def kernel(
    labels,
    features_old,
    features,
    outputs_old,
    outputs=None,
    prototypes=None,
    num_class=21,
    num_old_class=16,
    num_new_class=5,
    epoch=1,
    train_step=1,
    len_epoch=100,
):
    r = _get_runner()
    r.stage(make_in_maps(labels, features_old, features, outputs_old))
    out = r.execute()
    counts, sum_a, sum_o = combine_results(r.results(out))
    return host_finish(counts, sum_a, sum_o)

